# revision 3
# baseline (speedup 1.0000x reference)
"""Trainium2 Bass kernel for 8-head causal MultiHeadAttention (fp8 DoubleRow).

Problem (hardcoded): B=8, S=1024, d_model=512, H=8, d_k=128, d_v=256,
causal sequence mask, all-ones padding mask, zero biases, fp32 I/O,
rel-err budget 2e-2. Batch-parallel: one batch element per NeuronCore.

Core scheme:
  - All heavy matmuls run in fp8e4m3 with MatmulPerfMode.DoubleRow: each
    instruction contracts TWO 128-deep k-subtiles at 0.5 cycles/row (4x the
    fp32r rate). Precision is managed by power-of-2 prescaling (dodging
    fp8's subnormal floor) and residual (x ~= x8 + r8) operand pairs on the
    value path, where quantization error would pass straight to the output.
  - Scores are computed transposed S^T[t,q] per 128-row t-tile over its true
    causal window. The DoubleRow pair slot of the scores matmul carries the
    causal mask: lhsT = [kp_tile | 240*I], rhs = [qp window | -240*tril
    strip], so masked scores get -57600 added pre-exp at zero extra cycles.
  - exp runs on ACT with the operand prescales folded into its scale and a
    -0.6 bias recentering p into fp8's higher-resolution band (softmax is
    invariant to the constant factor).
  - P@V uses lhsT = [vp8 | vp8_residual] pairs against a stride-0 rhs pair
    (pt streamed twice): ~bf16-accurate values at fp8-pair speed. Row sums
    ride a [1|1] ones pair whose value 32 folds the denominators' scale.
  - Eviction flow:
  - oT is stored fp16 (no residual pair): the normalize multiply writes fp16
    directly, and the output projection runs plain fp16 matmuls.
  - V-projection PSUM is evicted once to fp32 SBUF; the fp8 + residual pair
    split runs on the (otherwise idle) GPSIMD engine, which cannot touch PSUM.
  - Projection/V PSUM tiles are packed two-per-bank so evictions are 512 wide.
  - Scores are emitted two tiles ahead of PV so the ACT exp stream never
    starves the PE.

Engine budget (8 heads): PE ~85us, ACT ~58, DVE ~63, Pool ~52.
"""

import numpy as np
import ml_dtypes

import concourse.bacc as bacc
import concourse.mybir as mybir
from concourse import tile
from concourse.ap import AP
from concourse.bass_utils import run_bass_kernel_spmd

B, S, D, H, DK, DV = 8, 1024, 512, 8, 128, 256
F32 = mybir.dt.float32
F16 = mybir.dt.float16
FP8 = mybir.dt.float8e4
ACT = mybir.ActivationFunctionType
ALU = mybir.AluOpType
DR = mybir.MatmulPerfMode.DoubleRow
# NOTE: mybir.dt.float8e4 == ml_dtypes.float8_e4m3 (IEEE variant, max 240).
E4 = ml_dtypes.float8_e4m3
FP8_MAX = 240.0

EXP_SCALE = float(np.float32(1.0 / (np.sqrt(np.float32(DK)) * 256.0)))
EXP_BIAS = -0.6
QP_W, STRIP_W = 1024 + 640, 640   # qp tile: [qp8 | tril strip]
KP_W = 1024 + 128                 # kp tile: [kp8 | FP8_MAX*I]

_CACHE = {}


def _pairap(t, base_off, sub1_off, n):
    """[128, 2, n] AP on tile t: sub0 at col base_off, sub1 at sub1_off."""
    a = t[:]
    return AP(
        a.tensor,
        a.offset + base_off,
        [list(a.ap[0]), [sub1_off - base_off, 2], [1, n]],
    )


def _zerostride(t, n):
    """[128, 2, n] AP streaming tile t's first n cols twice (dim1 stride 0)."""
    a = t[:]
    return AP(a.tensor, a.offset, [list(a.ap[0]), [0, 2], [1, n]])


def build():
    nc = bacc.Bacc(trn_type="TRN2", target_bir_lowering=False, debug=False)

    qT_d = nc.dram_tensor("qT8", [D, S], FP8, kind="ExternalInput").ap()
    kT_d = nc.dram_tensor("kT8", [D, S], FP8, kind="ExternalInput").ap()
    vT_d = nc.dram_tensor("vT8", [D, S], FP8, kind="ExternalInput").ap()
    vTr_d = nc.dram_tensor("vTr8", [D, S], FP8, kind="ExternalInput").ap()
    wq_d = nc.dram_tensor("wq8", [H, D, DK], FP8, kind="ExternalInput").ap()
    wk_d = nc.dram_tensor("wk8", [H, D, DK], FP8, kind="ExternalInput").ap()
    wv_d = nc.dram_tensor("wv8", [H, D, DV], FP8, kind="ExternalInput").ap()
    wvr_d = nc.dram_tensor("wvr8", [H, D, DV], FP8, kind="ExternalInput").ap()
    wo_d = nc.dram_tensor("wo16", [H * DV, D], F16, kind="ExternalInput").ap()
    strip_d = nc.dram_tensor("strip8", [128, STRIP_W], FP8, kind="ExternalInput").ap()
    imax_d = nc.dram_tensor("imax8", [128, 128], FP8, kind="ExternalInput").ap()
    ones_d = nc.dram_tensor("ones2", [128, 256], FP8, kind="ExternalInput").ap()
    outT_d = nc.dram_tensor("outT", [D, S], F16, kind="ExternalOutput").ap()

    with tile.TileContext(nc) as tc:
        with (
            tc.tile_pool(name="const", bufs=1) as const,
            tc.tile_pool(name="whead", bufs=2) as whead,
            tc.tile_pool(name="projp", bufs=2) as projp,
            tc.tile_pool(name="vp32p", bufs=2) as vp32p,
            tc.tile_pool(name="vpp", bufs=2) as vpp,
            tc.tile_pool(name="ptp", bufs=6) as ptp,
            tc.tile_pool(name="pbsp", bufs=2) as pbsp,
            tc.tile_pool(name="oallp", bufs=1) as oallp,
            tc.tile_pool(name="wop", bufs=16) as wop,
            tc.tile_pool(name="outst", bufs=4) as outst,
        ):
            _cm_pp = tc.tile_pool(name="ps_p", bufs=2, space="PSUM")
            ps_p = _cm_pp.__enter__()
            _cm_ps = tc.tile_pool(name="ps_s", bufs=3, space="PSUM")
            ps_s = _cm_ps.__enter__()
            _cm_pa = tc.tile_pool(name="ps_a", bufs=3, space="PSUM")
            ps_a = _cm_pa.__enter__()

            # ---- ACT table warmup + statics + inputs (startup-ordered) ----
            bias_t = const.tile([128, 1], F32, tag="biast")
            nc.any.memset(bias_t[:], EXP_BIAS)
            warm = const.tile([128, 1], F32, tag="actwarm")
            nc.any.memset(warm[:], 0.0)
            nc.scalar.activation(warm[:], warm[:], ACT.Exp, bias=bias_t[:])

            def load_head_weights(h):
                wq_s = whead.tile([128, 4 * DK], FP8, tag="wq", name=f"wq{h}")
                nc.sync.dma_start(
                    wq_s[:].rearrange("p (k m) -> p k m", k=4),
                    wq_d[h].rearrange("(k p) m -> p k m", p=128),
                )
                wk_s = whead.tile([128, 4 * DK], FP8, tag="wk", name=f"wk{h}")
                nc.sync.dma_start(
                    wk_s[:].rearrange("p (k m) -> p k m", k=4),
                    wk_d[h].rearrange("(k p) m -> p k m", p=128),
                )
                wv_s = whead.tile([128, 4 * DV], FP8, tag="wv", name=f"wv{h}")
                nc.sync.dma_start(
                    wv_s[:].rearrange("p (k m) -> p k m", k=4),
                    wv_d[h].rearrange("(k p) m -> p k m", p=128),
                )
                wvr_s = whead.tile([128, 4 * DV], FP8, tag="wvr", name=f"wvr{h}")
                nc.sync.dma_start(
                    wvr_s[:].rearrange("p (k m) -> p k m", k=4),
                    wvr_d[h].rearrange("(k p) m -> p k m", p=128),
                )
                return wq_s, wk_s, wv_s, wvr_s

            # input tensors as [128, 4(k), 1024] fp8; q/k split by column halves
            # so the first projection chunk can start early
            qTs = const.tile([128, 4 * S], FP8, tag="qT")
            kTs = const.tile([128, 4 * S], FP8, tag="kT")
            vTs = const.tile([128, 4 * S], FP8, tag="vT")
            vTrs = const.tile([128, 4 * S], FP8, tag="vTr")

            qTr = qTs[:].rearrange("p (k q) -> p k q", k=4)
            kTr = kTs[:].rearrange("p (k q) -> p k q", k=4)
            vTr_ = vTs[:].rearrange("p (k q) -> p k q", k=4)
            vTrr = vTrs[:].rearrange("p (k q) -> p k q", k=4)

            def half_dma(dst_r, src_d, half):
                nc.sync.dma_start(
                    dst_r[:, :, 512 * half : 512 * half + 512],
                    src_d.rearrange("(k p) q -> p k q", p=128)[
                        :, :, 512 * half : 512 * half + 512
                    ],
                )

            # DMA order matches first-head consumption order
            wq0 = whead.tile([128, 4 * DK], FP8, tag="wq", name="wq0")
            nc.sync.dma_start(
                wq0[:].rearrange("p (k m) -> p k m", k=4),
                wq_d[0].rearrange("(k p) m -> p k m", p=128),
            )
            half_dma(qTr, qT_d, 0)
            wk0 = whead.tile([128, 4 * DK], FP8, tag="wk", name="wk0")
            nc.sync.dma_start(
                wk0[:].rearrange("p (k m) -> p k m", k=4),
                wk_d[0].rearrange("(k p) m -> p k m", p=128),
            )
            half_dma(kTr, kT_d, 0)
            wv0 = whead.tile([128, 4 * DV], FP8, tag="wv", name="wv0")
            nc.sync.dma_start(
                wv0[:].rearrange("p (k m) -> p k m", k=4),
                wv_d[0].rearrange("(k p) m -> p k m", p=128),
            )
            wvr0 = whead.tile([128, 4 * DV], FP8, tag="wvr", name="wvr0")
            nc.sync.dma_start(
                wvr0[:].rearrange("p (k m) -> p k m", k=4),
                wvr_d[0].rearrange("(k p) m -> p k m", p=128),
            )
            half_dma(vTr_, vT_d, 0)
            half_dma(vTrr, vTr_d, 0)
            ones2 = const.tile([128, 256], FP8, tag="ones2")
            nc.sync.dma_start(ones2[:], ones_d)
            half_dma(qTr, qT_d, 1)
            half_dma(kTr, kT_d, 1)
            half_dma(vTr_, vT_d, 1)
            half_dma(vTrr, vTr_d, 1)
            weights = {0: (wq0, wk0, wv0, wvr0)}

            oTall = oallp.tile([128, 16 * S], F16, tag="oTall")

            def proj_units(h, w):
                """Returns (qp, kp, vp, units): units = list of closures, each
                emitting one psum-bank's worth of projection work. Interleaved
                into the previous head's attention loop as PE filler."""
                wq_s, wk_s, wv_s, wvr_s = w
                qp = projp.tile([128, QP_W], FP8, tag="qp", name=f"qp{h}")
                kp = projp.tile([128, KP_W], FP8, tag="kp", name=f"kp{h}")
                vp32 = vp32p.tile([128, 8 * DV], F32, tag="vp32", name=f"vp32{h}")
                vp = vpp.tile([128, 8 * 2 * DV], FP8, tag="vp", name=f"vp{h}")
                if h < 2:
                    nc.sync.dma_start(qp[:, 1024 : 1024 + STRIP_W], strip_d)
                    nc.sync.dma_start(kp[:, 1024:1152], imax_d)
                units = []

                def qk_unit(dst, w_s, src, half, eng):
                    def emit():
                        wr = w_s[:].rearrange("p (k m) -> p k m", k=4)
                        sr = src[:].rearrange("p (k q) -> p k q", k=4)
                        p = ps_p.tile([128, 512], F32, tag="pp")
                        for c in range(2):
                            sl = slice(512 * half + 256 * c, 512 * half + 256 * c + 256)
                            for kk in range(2):
                                nc.tensor.matmul(
                                    p[:, 256 * c : 256 * c + 256],
                                    wr[:, 2 * kk : 2 * kk + 2, :],
                                    sr[:, 2 * kk : 2 * kk + 2, sl],
                                    start=(kk == 0),
                                    stop=(kk == 1),
                                    perf_mode=DR,
                                    skip_group_check=True,
                                )
                        dsl = dst[:, 512 * half : 512 * half + 512]
                        if eng == "v":
                            nc.vector.tensor_scalar_mul(dsl, p[:], 0.25)
                        else:
                            nc.scalar.activation(dsl, p[:], ACT.Copy, scale=0.25)
                    return emit

                def v_unit(ii, eng):
                    def emit():
                        wvb = wv_s[:].rearrange("p (k m) -> p k m", k=4)
                        wvr = wvr_s[:].rearrange("p (k m) -> p k m", k=4)
                        vb = vTs[:].rearrange("p (k q) -> p k q", k=4)
                        vr = vTrs[:].rearrange("p (k q) -> p k q", k=4)
                        vpr = vp[:].rearrange("p (i s m) -> p i s m", i=8, s=2)
                        p = ps_p.tile([128, 512], F32, tag="pp")
                        for sub in range(2):
                            i = 2 * ii + sub
                            first = True
                            for lhs_r, rhs_r in ((vb, wvb), (vb, wvr), (vr, wvb)):
                                for kk in range(2):
                                    nc.tensor.matmul(
                                        p[:, 256 * sub : 256 * sub + 256],
                                        lhs_r[:, 2 * kk : 2 * kk + 2,
                                              128 * i : 128 * i + 128],
                                        rhs_r[:, 2 * kk : 2 * kk + 2, :],
                                        start=first,
                                        stop=(lhs_r is vr and kk == 1),
                                        perf_mode=DR,
                                        skip_group_check=True,
                                    )
                                    first = False
                        dst32 = vp32[:, 512 * ii : 512 * ii + 512]
                        if eng == "v":
                            nc.vector.tensor_copy(dst32, p[:])
                        else:
                            nc.scalar.activation(dst32, p[:], ACT.Copy)
                        v8dst = vpr[:, 2 * ii : 2 * ii + 2, 0, :]
                        nc.gpsimd.tensor_copy(
                            v8dst, dst32.rearrange("p (s m) -> p s m", s=2)
                        )
                        nc.gpsimd.tensor_sub(
                            vpr[:, 2 * ii : 2 * ii + 2, 1, :],
                            dst32.rearrange("p (s m) -> p s m", s=2),
                            v8dst,
                        )
                    return emit

                u_list = [
                    qk_unit(qp, wq_s, qTs, 0, "v"),
                    qk_unit(kp, wk_s, kTs, 0, "a"),
                    qk_unit(qp, wq_s, qTs, 1, "v"),
                    qk_unit(kp, wk_s, kTs, 1, "a"),
                ] + [v_unit(ii, "v") for ii in range(4)]
                if h == 0:
                    u_list = [u_list[0], u_list[1], u_list[4], u_list[5],
                              u_list[2], u_list[3], u_list[6], u_list[7]]
                units.extend(u_list)
                return qp, kp, vp, units

            def attn(h, qp, kp, vp, filler):
                """filler: iterator of closures (next head's projection units)
                pulled between attention tiles to fill PE bubbles."""
                vpr = vp[:].rearrange("p (i s m) -> p i s m", i=8, s=2)
                # flat tile sequence across both q-chunks
                seq = [(0, i) for i in range(4)] + [(1, i) for i in range(8)]
                state = {}

                def emit_score(n):
                    j, i = seq[n]
                    wlo = max(0, 128 * i - 512 * j)
                    nw = 512 - wlo
                    diag = i >= 4 * j
                    psc = ps_s.tile([128, nw], F32, tag="ps", name=f"psc{h}_{j}_{i}")
                    lhsT = _pairap(kp, 128 * i, 1024, 128)
                    soff = 1024 + (0 if diag else 128)
                    rhs = _pairap(qp, 512 * j + wlo, soff, nw)
                    nc.tensor.matmul(psc[:], lhsT, rhs, start=True, stop=True,
                                     perf_mode=DR)
                    pt = ptp.tile([128, nw], FP8, tag="pt", name=f"pt{h}_{j}_{i}")
                    nc.scalar.activation(pt[:], psc[:], ACT.Exp,
                                         bias=bias_t[:], scale=EXP_SCALE)
                    state[n] = (pt, wlo)

                def emit_pv(n):
                    j, i = seq[n]
                    n_t = 4 * (j + 1)
                    if i == 0:
                        state["po", j] = [
                            ps_a.tile([128, 512], F32, tag="acc",
                                      name=f"po{h}_{j}_{vh}")
                            for vh in range(2)
                        ]
                        state["pr", j] = ps_a.tile([128, 512], F32, tag="acc",
                                                   name=f"pr{h}_{j}")
                    po, pr = state["po", j], state["pr", j]
                    pt, wlo = state.pop(n)
                    nw = 512 - wlo
                    pt2 = _zerostride(pt, nw)
                    for vh in range(2):
                        nc.tensor.matmul(
                            po[vh][:, wlo:512],
                            vpr[:, i, :, 128 * vh : 128 * vh + 128],
                            pt2,
                            start=(i == 0),
                            stop=(i == n_t - 1),
                            perf_mode=DR,
                            skip_group_check=True,
                        )
                    nc.tensor.matmul(
                        pr[:, wlo:512],
                        ones2[:].rearrange("p (s m) -> p s m", s=2),
                        pt2,
                        start=(i == 0),
                        stop=(i == n_t - 1),
                        perf_mode=DR,
                        skip_group_check=True,
                    )
                    if i == n_t - 1:
                        pbs = pbsp.tile([128, 512], F32, tag="pbs")
                        nc.vector.reciprocal(pbs[:], pr[:])
                        for vh in range(2):
                            kk = 2 * h + vh
                            nc.vector.tensor_mul(
                                oTall[:, S * kk + 512 * j : S * kk + 512 * j + 512],
                                po[vh][:],
                                pbs[:],
                            )

                emit_score(0)
                emit_score(1)
                for n in range(12):
                    if n + 2 < 12:
                        emit_score(n + 2)
                    emit_pv(n)
                    nxt = next(filler, None)
                    if nxt is not None:
                        nxt()
                for nxt in filler:
                    nxt()

            # software pipeline: head 0 starts attention after its first-half
            # projections (enough for q-chunk 0); the second half rides as
            # filler. Then the pipeline refills (proj1+proj2) and settles into
            # proj(h+1)-block-before-attn(h).
            wo_tiles = []

            def load_wo():
                for g in range(16):
                    w = wop.tile([128, D], F16, tag="wo", name=f"wo{g}")
                    nc.sync.dma_start(w[:], wo_d[128 * g : 128 * g + 128, :])
                    wo_tiles.append(w)

            proj = {0: proj_units(0, weights[0])}
            weights[1] = load_head_weights(1)
            for u in proj[0][3]:
                u()
            for h in range(H):
                if h + 2 < H:
                    weights[h + 2] = load_head_weights(h + 2)
                if h == 5:
                    load_wo()
                if h + 1 < H:
                    proj[h + 1] = proj_units(h + 1, weights[h + 1])
                    for u in proj[h + 1][3]:
                        u()
                qp_h, kp_h, vp_h, _ = proj.pop(h)
                attn(h, qp_h, kp_h, vp_h, iter(()))

            # ---- output projection (fp16, plain matmuls) ----
            oTr = oTall[:].rearrange("p (k q) -> p k q", k=16)
            _psp = [ps_p, ps_p, ps_s, ps_s, ps_s, ps_a, ps_a, ps_a]
            _tg = ["pp", "pp", "ps", "ps", "ps", "acc", "acc", "acc"]
            for ch in range(8):
                m, c = divmod(ch, 2)
                p = _psp[ch].tile([128, 512], F32, tag=_tg[ch], name=f"pout{ch}")
                for g in range(16):
                    nc.tensor.matmul(
                        p[:],
                        wo_tiles[g][:, 128 * m : 128 * m + 128],
                        oTr[:, g, 512 * c : 512 * c + 512],
                        start=(g == 0),
                        stop=(g == 15),
                        skip_group_check=True,
                    )
                st = outst.tile([128, 512], F16, tag="st")
                if ch % 2 == 0:
                    nc.scalar.activation(st[:], p[:], ACT.Copy)
                else:
                    nc.vector.tensor_copy(st[:], p[:])
                nc.sync.dma_start(
                    outT_d[128 * m : 128 * m + 128, 512 * c : 512 * c + 512], st[:]
                )

            _cm_pa.__exit__(None, None, None)
            _cm_ps.__exit__(None, None, None)
            _cm_pp.__exit__(None, None, None)

    nc.compile()
    return nc


def _q8(x):
    return np.ascontiguousarray(x).astype(E4)


def _prep(Q, K, V, padding_mask, sequence_mask, Wq, bq, Wk, bk, Wv, bv, Wo, bo):
    assert np.asarray(padding_mask).min() == 1, "kernel assumes all-ones padding mask"
    seq = np.asarray(sequence_mask)
    assert np.array_equal(seq, np.tril(np.ones((S, S), seq.dtype))), "causal mask"
    for bias in (bq, bk, bv, bo):
        assert np.abs(np.asarray(bias)).max() == 0.0, "kernel assumes zero biases"

    tril_strict = np.tril(np.ones((128, 128), np.float32), -1)
    strip = np.concatenate(
        [tril_strict * np.float32(-FP8_MAX), np.zeros((128, 512), np.float32)], axis=1
    )
    shared = {
        "wq8": _q8(np.asarray(Wq, np.float32) * 64.0),
        "wk8": _q8(np.asarray(Wk, np.float32) * 64.0),
        "strip8": _q8(strip),
        "imax8": _q8(np.eye(128, dtype=np.float32) * FP8_MAX),
        "ones2": _q8(np.full((128, 256), 32.0, np.float32)),
        "wo16": np.ascontiguousarray(np.asarray(Wo, np.float32).astype(np.float16)),
    }
    wv64 = np.asarray(Wv, np.float32) * 64.0
    wv8 = wv64.astype(E4)
    shared["wv8"] = np.ascontiguousarray(wv8)
    shared["wvr8"] = _q8(wv64 - wv8.astype(np.float32))

    in_maps = []
    for b in range(B):
        m = dict(shared)
        vT = np.asarray(V[b], np.float32).T
        vT8 = vT.astype(E4)
        m["qT8"] = _q8(np.asarray(Q[b], np.float32).T)
        m["kT8"] = _q8(np.asarray(K[b], np.float32).T)
        m["vT8"] = np.ascontiguousarray(vT8)
        m["vTr8"] = _q8(vT - vT8.astype(np.float32))
        in_maps.append(m)
    return in_maps


def kernel(Q, K, V, padding_mask, sequence_mask, Wq, bq, Wk, bk, Wv, bv, Wo, bo):
    if "nc" not in _CACHE:
        _CACHE["nc"] = build()
    nc = _CACHE["nc"]
    in_maps = _prep(Q, K, V, padding_mask, sequence_mask, Wq, bq, Wk, bk, Wv, bv, Wo, bo)
    res = run_bass_kernel_spmd(nc, in_maps, core_ids=list(range(B)))
    out = np.empty((B, S, D), np.float32)
    for b in range(B):
        out[b] = res.results[b]["outT"].T.astype(np.float32)
    return out
